# revision 9
# baseline (speedup 1.0000x reference)
"""ChildSum TreeLSTM (N=8192 nodes, 4-ary static heap tree, H=256, D=300) on 8 trn2 NeuronCores.

Strategy
--------
The tree is static: node i's children are 4i+1..4i+4 (clipped at N). The reverse
scan (children before parents) is equivalent to processing the tree level by
level, bottom-up; nodes within a level are independent, so each level is a
batched LSTM cell (matmuls + elementwise).

Sharding: the 256 level-4 subtrees are partitioned across the 8 cores (balanced
by the number of *internal* level-6 descendants, which determines level-7 leaf
count). Each core processes its forest fully locally — children of a sorted node
range are contiguous in the next level's sorted array, so the recurrence needs
no gathers and no cross-core communication. Cores output their 32 level-4 root
(h, c) states; the tiny top of the tree (levels 3..0, 85 nodes) plus the final
log_softmax run on the host in numpy.

On-device layout: everything is transposed — feature dim on SBUF partitions
(256 features = 2 halves of 128), nodes along the free axis. The child-h sums
and per-child forget gates then become strided slicing along the free axis, and
the x-side / h-side gate projections accumulate into the same PSUM tile.
Biases (bx + bh, zeros in practice) are folded into an extra ones-row of the
x-side matmul, so pad columns (zero x) self-compute to h = c = 0.
"""

import numpy as np

N = 8192
H = 256
D = 300
K = 4
OUT = 4
NCORES = 8
L7P = 384           # padded level-7 columns per core (4 * IPMAX)
IPMAX = 96          # max internal level-6 nodes per core
KDIM = 304          # padded contraction rows of xt/wx (300 emb + 1 ones + pad)
KUSE = 301          # rows actually used in matmuls
XCOLS = L7P + 512 + 128 + 32   # 1056 per-core node columns: [L7 | L6 | L5 | L4]

GATE_MAP = [0, 2, 3, 1]  # our gate order [i, o, u, f] -> reference gate indices

F32 = np.float32


def _build_plan():
    """Assign the 256 level-4 subtrees to 8 cores; build per-core column maps."""
    # w(u) = number of internal (has-children) level-6 descendants of L4 node u.
    # Full-weight subtrees (w=16) are u in [85, 127); u=127 has w=11; rest 0.
    full = list(range(85, 127))                               # 42 subtrees
    lights = list(range(128, 341))                            # 213 subtrees
    heavy_counts = [6, 6, 5, 5, 5, 5, 5, 5]                   # sums to 42
    light_counts = [26, 26, 26, 27, 27, 27, 27, 27]           # sums to 213
    cores = []
    hpos = 0
    lpos = 0
    for c in range(NCORES):
        hs = full[hpos:hpos + heavy_counts[c]]
        hpos += heavy_counts[c]
        if c == 2:
            hs = hs + [127]                                   # w sums: 96,96,91,80*5
        ls = lights[lpos:lpos + light_counts[c]]
        lpos += light_counts[c]
        cores.append(sorted(hs + ls))
    all_l4 = sorted(u for cs in cores for u in cs)
    assert all_l4 == list(range(85, 341)), "L4 assignment must partition [85, 341)"

    plan = []
    for c in range(NCORES):
        l4 = cores[c]
        assert len(l4) == 32
        l5 = [4 * u + 1 + k for u in l4 for k in range(K)]
        l6 = [4 * v + 1 + k for v in l5 for k in range(K)]
        wc = sum(1 for x in l6 if x < 2048)
        assert wc <= IPMAX
        l7 = []
        for x in l6[:wc]:
            for k in range(K):
                ch = 4 * x + 1 + k
                l7.append(ch if ch < N else -1)
        l7 += [-1] * (L7P - len(l7))
        cols = np.array(l7 + l6 + l5 + l4, dtype=np.int64)
        assert cols.shape == (XCOLS,)
        plan.append((cols, wc, np.array(l4, dtype=np.int64)))
    return plan


_PLAN = _build_plan()

# chunk schedule: (xoff, ncols, ip, child_level, out_level, out_off)
# child/out levels refer to state buffers keyed 7, 6, 5, 4.
_CHUNKS = [
    (0,    256, 0,   None, 7, 0),     # L7 leaves, part A
    (256,  128, 0,   None, 7, 256),   # L7 leaves, part B
    (640,  256, 0,   None, 6, 256),   # L6 leaf-only half
    (384,  256, 96,  7,    6, 0),     # L6 internal half (ip=96 of 256)
    (896,  128, 128, 6,    5, 0),     # L5 (all internal)
    (1024, 32,  32,  5,    4, 0),     # L4 (all internal)
]
_STATE_COLS = {7: L7P, 6: 512, 5: 128, 4: 32}


def _static_tree():
    idx = np.arange(N)[:, None] * K + 1 + np.arange(K)[None, :]
    mask = (idx < N).astype(F32)
    idx = np.where(idx < N, idx, 0).astype(np.int32)
    return idx, mask


_STATIC_IDX, _STATIC_MASK = _static_tree()


def _pack_weights(Wx, bx, Wh, bh):
    wx = np.zeros((KDIM, 4 * H), dtype=F32)
    for g, rg in enumerate(GATE_MAP):
        wx[:D, H * g:H * (g + 1)] = np.asarray(Wx[rg], dtype=F32).T
        wx[D, H * g:H * (g + 1)] = np.asarray(bx[rg], dtype=F32) + np.asarray(bh[rg], dtype=F32)
    wh = np.zeros((H, 3 * H), dtype=F32)
    for g, rg in enumerate([0, 2, 3]):  # i, o, u
        wh[:, H * g:H * (g + 1)] = np.asarray(Wh[rg], dtype=F32).T
    whf = np.ascontiguousarray(np.asarray(Wh[1], dtype=F32).T)
    return wx, wh, whf


def _pack_xt(xs, emb_table):
    X = np.asarray(emb_table, dtype=F32)[np.asarray(xs)]
    xts = []
    for cols, _, _ in _PLAN:
        xt = np.zeros((KDIM, XCOLS), dtype=F32)
        real = cols >= 0
        xt[:D, real] = X[cols[real]].T
        xt[D, real] = 1.0
        xts.append(xt)
    return xts


def _sigmoid(x):
    return (1.0 / (1.0 + np.exp(-x))).astype(F32)


def _host_top(Hbuf, Cbuf, xs, emb_table, Wx, bx, Wh, bh):
    """Compute tree levels 3..0 (nodes 0..84) on the host, numpy fp32."""
    Wx = np.asarray(Wx, dtype=F32)
    bx = np.asarray(bx, dtype=F32)
    Wh = np.asarray(Wh, dtype=F32)
    bh = np.asarray(bh, dtype=F32)
    emb = np.asarray(emb_table, dtype=F32)
    xs = np.asarray(xs)
    for lo, hi in [(21, 85), (5, 21), (1, 5), (0, 1)]:
        ids = np.arange(lo, hi)
        Xl = emb[xs[ids]]                                   # [n, D]
        gx = np.einsum('ghd,nd->ngh', Wx, Xl).astype(F32) + bx
        cidx = ids[:, None] * K + 1 + np.arange(K)[None, :]  # all valid (< 341)
        Hc = Hbuf[cidx]
        Cc = Cbuf[cidx]
        hs = Hc.sum(1)
        ig = _sigmoid(gx[:, 0] + hs @ Wh[0].T + bh[0])
        og = _sigmoid(gx[:, 2] + hs @ Wh[2].T + bh[2])
        ug = np.tanh(gx[:, 3] + hs @ Wh[3].T + bh[3]).astype(F32)
        f = _sigmoid(gx[:, 1][:, None, :] + Hc @ Wh[1].T + bh[1])
        cc = ig * ug + (f * Cc).sum(1)
        hh = og * np.tanh(cc).astype(F32)
        Hbuf[ids] = hh
        Cbuf[ids] = cc
    return Hbuf[0]


def _log_softmax(x):
    m = np.max(x)
    e = np.exp(x - m)
    return (x - m - np.log(e.sum())).astype(F32)


def simulate_cores_numpy(inputs):
    """Numpy emulation of the exact device data layout & chunk schedule.

    Returns (Hbuf, Cbuf) filled for nodes [85, 341) — for validating the plan
    against the reference without hardware.
    """
    xs = np.asarray(inputs["xs"])
    wx, wh, whf = _pack_weights(inputs["Wx"], inputs["bx"], inputs["Wh"], inputs["bh"])
    xts = _pack_xt(xs, inputs["emb_table"])
    Hbuf = np.zeros((341, H), dtype=F32)
    Cbuf = np.zeros((341, H), dtype=F32)
    for c in range(NCORES):
        cols, wc, l4 = _PLAN[c]
        xt = xts[c]
        state_h = {lv: np.zeros((H, n), dtype=F32) for lv, n in _STATE_COLS.items()}
        state_c = {lv: np.zeros((H, n), dtype=F32) for lv, n in _STATE_COLS.items()}
        for (xoff, nc_, ip, child, outlv, ooff) in _CHUNKS:
            xk = xt[:KUSE, xoff:xoff + nc_]                     # [301, nc]
            G = wx[:KUSE].T @ xk                                # [1024, nc]
            gi = G[0:H]
            go = G[H:2 * H]
            gu = G[2 * H:3 * H]
            gf = G[3 * H:4 * H]
            if ip > 0:
                ch_h = state_h[child][:, :4 * ip]               # [H, 4ip]
                ch_c = state_c[child][:, :4 * ip]
                hs = ch_h.reshape(H, ip, K).sum(axis=2)         # [H, ip]
                A = wh.T @ hs                                   # [768, ip]
                gi[:, :ip] += A[0:H]
                go[:, :ip] += A[H:2 * H]
                gu[:, :ip] += A[2 * H:3 * H]
                Fp = whf.T @ ch_h                               # [H, 4ip]
                FA = Fp + np.repeat(gf[:, :ip], K, axis=1)
                FS = _sigmoid(FA) * ch_c
                csum = FS.reshape(H, ip, K).sum(axis=2)
            ig = _sigmoid(gi)
            og = _sigmoid(go)
            ug = np.tanh(gu).astype(F32)
            cc = ig * ug
            if ip > 0:
                cc[:, :ip] += csum
            hh = og * np.tanh(cc).astype(F32)
            state_h[outlv][:, ooff:ooff + nc_] = hh
            state_c[outlv][:, ooff:ooff + nc_] = cc
        Hbuf[l4] = state_h[4].T
        Cbuf[l4] = state_c[4].T
    return Hbuf, Cbuf


# ----------------------------------------------------------------------------
# Bass device program
# ----------------------------------------------------------------------------

_COMPILED = None


def _build_device_program():
    import concourse.bacc as bacc
    import concourse.tile as tile
    import concourse.mybir as mybir
    import concourse.bass as bass

    f32 = mybir.dt.float32
    f32r = mybir.dt.float32r
    Sig = mybir.ActivationFunctionType.Sigmoid
    Tanh = mybir.ActivationFunctionType.Tanh

    def mm(out, lhsT, rhs, **kw):
        # float32r: same fp32 bytes, single-pass reduced-precision multiply
        # (vs fp32's two half-speed passes + double weight load).
        nc.tensor.matmul(out, lhsT, rhs, **kw)

    nc = bacc.Bacc("TRN2", target_bir_lowering=False, debug=False,
                   num_devices=NCORES)

    xt_d = nc.dram_tensor("xt", [KDIM, XCOLS], f32r, kind="ExternalInput")
    wx_d = nc.dram_tensor("wx", [KDIM, 4 * H], f32r, kind="ExternalInput")
    wh_d = nc.dram_tensor("wh", [H, 3 * H], f32r, kind="ExternalInput")
    whf_d = nc.dram_tensor("whf", [H, H], f32r, kind="ExternalInput")
    out_h_d = nc.dram_tensor("out_h", [128, 2, 32], f32r, kind="ExternalOutput")
    out_c_d = nc.dram_tensor("out_c", [128, 2, 32], f32, kind="ExternalOutput")

    with tile.TileContext(nc) as tc:
        import contextlib
        with contextlib.ExitStack() as ctx:
            inp = ctx.enter_context(tc.tile_pool(name="inp", bufs=1))
            st = ctx.enter_context(tc.tile_pool(name="state", bufs=1))
            wk = ctx.enter_context(tc.tile_pool(name="work", bufs=2))
            fwk = ctx.enter_context(tc.tile_pool(name="fwork", bufs=3))
            ps = ctx.enter_context(
                tc.tile_pool(name="psum", bufs=2, space="PSUM"))

            # --- inputs to SBUF (weights first; xt split per chunk range so
            # the first chunk's matmuls start as soon as its columns land)
            xt_s = []
            wx_s = []
            for k, (r0, r1) in enumerate([(0, 128), (128, 256), (256, 304)]):
                t = inp.tile([r1 - r0, 4 * H], f32r, tag=f"wx{k}")
                nc.sync.dma_start(out=t[:], in_=wx_d[r0:r1, :])
                wx_s.append(t)
            wh_s = []
            whf_s = []
            for k, (r0, r1) in enumerate([(0, 128), (128, 256)]):
                t = inp.tile([128, 3 * H], f32r, tag=f"wh{k}")
                nc.sync.dma_start(out=t[:], in_=wh_d[r0:r1, :])
                wh_s.append(t)
                t = inp.tile([128, H], f32r, tag=f"whf{k}")
                nc.sync.dma_start(out=t[:], in_=whf_d[r0:r1, :])
                whf_s.append(t)
            for k, (r0, r1) in enumerate([(0, 128), (128, 256), (256, 304)]):
                t = inp.tile([r1 - r0, XCOLS], f32r, tag=f"xt{k}")
                xt_s.append(t)
            for (xoff, cn, _ip, _ch, _ol, _oo) in _CHUNKS:
                for k, (r0, r1) in enumerate([(0, 128), (128, 256), (256, 304)]):
                    nc.sync.dma_start(
                        out=xt_s[k][:, xoff:xoff + cn],
                        in_=xt_d[r0:r1, xoff:xoff + cn])

            # --- persistent state tiles
            SH = {lv: st.tile([128, 2, n], f32r, tag=f"h{lv}", name=f"sh{lv}")
                  for lv, n in _STATE_COLS.items()}
            SC = {lv: st.tile([128, 2, n], f32, tag=f"c{lv}", name=f"sc{lv}")
                  for lv, n in _STATE_COLS.items()}

            krows = [(0, 128), (128, 256), (256, KUSE)]

            for (xoff, cn, ip, child, outlv, ooff) in _CHUNKS:
                # hs = sum of 4 child h columns (only for internal cols)
                hs = None
                if ip > 0:
                    hs = wk.tile([128, 2, ip], f32r, tag="hs")
                    for phi in range(2):
                        cv = SH[child][:, phi, 0:4 * ip].rearrange(
                            "p (n k) -> p n k", k=K)
                        dst = hs[:, phi, :]
                        nc.vector.tensor_add(dst, cv[:, :, 0], cv[:, :, 1])
                        nc.vector.tensor_add(dst, dst, cv[:, :, 2])
                        nc.vector.tensor_add(dst, dst, cv[:, :, 3])

                # i, o, u gate pre-activations: x-side + h-side into one PSUM
                gates = []
                for gi_, func in ((0, Sig), (1, Sig), (2, Tanh)):
                    P = ps.tile([128, 2, cn], f32, tag=f"p{gi_}")
                    for phi in range(2):
                        col = H * gi_ + 128 * phi
                        for k in range(3):
                            r0, r1 = krows[k]
                            mm(
                                P[:, phi, :],
                                wx_s[k][0:r1 - r0, col:col + 128],
                                xt_s[k][0:r1 - r0, xoff:xoff + cn],
                                start=(k == 0),
                                stop=(k == 2 and ip == 0),
                            )
                        if ip > 0:
                            for k in range(2):
                                nc.tensor.matmul(
                                    P[:, phi, 0:ip],
                                    wh_s[k][:, col:col + 128],
                                    hs[:, k, 0:ip],
                                    start=False,
                                    stop=(k == 1),
                                    skip_group_check=True,
                                )
                    G = wk.tile([128, 2, cn], f32, tag=f"g{gi_}")
                    nc.scalar.activation(G[:], P[:], func)
                    gates.append(G)
                IG, OG, UG = gates

                csum = None
                if ip > 0:
                    # f = sigmoid(gf + Whf @ h_child): the per-node gf is
                    # broadcast over the 4 children directly in the matmul via
                    # a step-0 rhs access pattern (each xt column streamed 4x).
                    csum = wk.tile([128, 2, ip], f32, tag="csum")
                    for phi in range(2):
                        col = 3 * H + 128 * phi
                        Pfc = ps.tile([128, 4 * ip], f32, tag="pf")
                        for k in range(2):
                            mm(
                                Pfc[:],
                                whf_s[k][:, 128 * phi:128 * phi + 128],
                                SH[child][:, k, 0:4 * ip],
                                start=(k == 0),
                                stop=False,
                            )
                        for k in range(3):
                            r0, r1 = krows[k]
                            rhs = xt_s[k][0:r1 - r0, xoff:xoff + ip][:, :, None]
                            rhs = rhs.broadcast_to([r1 - r0, ip, K])
                            mm(
                                Pfc[:],
                                wx_s[k][0:r1 - r0, col:col + 128],
                                rhs,
                                start=False,
                                stop=(k == 2),
                                skip_group_check=True,
                            )
                        FS = fwk.tile([128, 4 * ip], f32, tag="fs")
                        nc.scalar.activation(FS[:], Pfc[:], Sig)
                        nc.vector.tensor_mul(
                            FS[:], FS[:], SC[child][:, phi, 0:4 * ip])
                        sv = FS.rearrange("p (n k) -> p n k", k=K)
                        dst = csum[:, phi, :]
                        nc.vector.tensor_add(dst, sv[:, :, 0], sv[:, :, 1])
                        nc.vector.tensor_add(dst, dst, sv[:, :, 2])
                        nc.vector.tensor_add(dst, dst, sv[:, :, 3])

                # c = ig*ug (+ csum on internal cols); h = og*tanh(c)
                Cdst = SC[outlv][:, :, ooff:ooff + cn]
                nc.vector.tensor_mul(Cdst, IG[:], UG[:])
                if ip > 0:
                    nc.vector.tensor_add(
                        SC[outlv][:, :, ooff:ooff + ip],
                        SC[outlv][:, :, ooff:ooff + ip],
                        csum[:],
                    )
                TC = wk.tile([128, 2, cn], f32, tag="tc")
                nc.scalar.activation(TC[:], Cdst, Tanh)
                nc.vector.tensor_mul(
                    SH[outlv][:, :, ooff:ooff + cn], OG[:], TC[:])

            nc.sync.dma_start(out=out_h_d[:], in_=SH[4][:])
            nc.sync.dma_start(out=out_c_d[:], in_=SC[4][:])

    nc.compile()
    return nc


def _get_compiled():
    global _COMPILED
    if _COMPILED is None:
        _COMPILED = _build_device_program()
    return _COMPILED


def _numpy_fallback(xs, child_idx, child_mask, emb_table, Wx, bx, Wh, bh,
                    Wout, bout):
    """Exact sequential scan in numpy; only used if the tree is not the
    expected static 4-ary heap."""
    X = np.asarray(emb_table, dtype=F32)[np.asarray(xs)]
    Wx = np.asarray(Wx, dtype=F32)
    Wh = np.asarray(Wh, dtype=F32)
    bx = np.asarray(bx, dtype=F32)
    bh = np.asarray(bh, dtype=F32)
    gx = np.einsum('ghd,nd->ngh', Wx, X).astype(F32) + bx
    Hb = np.zeros((N, H), dtype=F32)
    Cb = np.zeros((N, H), dtype=F32)
    ci = np.asarray(child_idx)
    cm = np.asarray(child_mask, dtype=F32)
    for i in range(N - 1, -1, -1):
        idx = ci[i]
        m = cm[i][:, None]
        Hc = Hb[idx] * m
        Cc = Cb[idx] * m
        hs = Hc.sum(0)
        g = gx[i]
        ig = _sigmoid(g[0] + Wh[0] @ hs + bh[0])
        og = _sigmoid(g[2] + Wh[2] @ hs + bh[2])
        ug = np.tanh(g[3] + Wh[3] @ hs + bh[3]).astype(F32)
        f = _sigmoid(g[1] + Hc @ Wh[1].T + bh[1])
        c = ig * ug + (f * Cc).sum(0)
        Hb[i] = og * np.tanh(c).astype(F32)
        Cb[i] = c
    logits = np.asarray(Wout, dtype=F32) @ Hb[0] + np.asarray(bout, dtype=F32)
    return _log_softmax(logits)


def kernel(xs, child_idx, child_mask, emb_table, Wx, bx, Wh, bh, Wout, bout):
    xs = np.asarray(xs)
    if not (np.array_equal(np.asarray(child_idx), _STATIC_IDX)
            and np.array_equal(np.asarray(child_mask, dtype=F32), _STATIC_MASK)):
        return _numpy_fallback(xs, child_idx, child_mask, emb_table, Wx, bx,
                               Wh, bh, Wout, bout)

    from concourse.bass_utils import run_bass_kernel_spmd

    wx, wh, whf = _pack_weights(Wx, bx, Wh, bh)
    xts = _pack_xt(xs, emb_table)
    in_maps = [
        {"xt": xts[c], "wx": wx, "wh": wh, "whf": whf} for c in range(NCORES)
    ]
    nc = _get_compiled()
    res = run_bass_kernel_spmd(nc, in_maps, core_ids=list(range(NCORES)))

    Hbuf = np.zeros((341, H), dtype=F32)
    Cbuf = np.zeros((341, H), dtype=F32)
    for c in range(NCORES):
        _, _, l4 = _PLAN[c]
        oh = res.results[c]["out_h"]   # [128, 2, 32]
        oc = res.results[c]["out_c"]
        Hbuf[l4] = np.concatenate([oh[:, 0, :], oh[:, 1, :]], axis=0).T
        Cbuf[l4] = np.concatenate([oc[:, 0, :], oc[:, 1, :]], axis=0).T

    h0 = _host_top(Hbuf, Cbuf, xs, emb_table, Wx, bx, Wh, bh)
    logits = np.asarray(Wout, dtype=F32) @ h0 + np.asarray(bout, dtype=F32)
    return _log_softmax(logits)
